# revision 7
# baseline (speedup 1.0000x reference)
"""Contrastive loss kernel for Trainium2, 8 NeuronCores (SPMD).

Math (matches the reference):
    z = concat(normalize(z_i), normalize(z_j))        # (2B, D) = (8192, 256)
    sim = (z @ z.T) / T
    positives[g] = sim[g, (g+B) mod 2B]               # (2B,)
    neg_max[g] = max_{j != g} sim[g, j]
    loss = mean(neg_max) - logsumexp(positives)       # scalar

Sharding: data-parallel over rows. Core k receives z rolled by -1024*k so its
band is always rows [0, 1024) of its local copy -> identical static program on
every core (diagonal / positive blocks land at fixed tile offsets).

Device pipeline per core (v4):
  f32 chunk loads -> ACT squares + DVE windowed reduce give row norms ->
  sqrt + recip -> DVE tensor_scalar fused scale+downcast to bf16 ->
  store normalized bf16 to DRAM scratch -> DMA xbar transpose loads build
  zT [d, row] (no compute engines) -> per 128-row block: 4 psum quads
  (8 matmuls each), diag masked / positives extracted on psum early, ACT
  evacuates quads to bf16 candidates, DVE deep-folds each block's 8192-wide
  candidate row (2x bf16 max tree), with folds deferred one block so they
  never gate the next block's psum ops.
Host: gather, divide by T, mean/LSE in float64, return float32 scalar.
"""

import numpy as np

TEMPERATURE = 0.1
B, D = 4096, 256
R = 2 * B                # 8192 total rows
NCORES = 8
MROWS = R // NCORES      # 1024 rows per core
P = 128                  # SBUF partitions
NT_ROW = R // P          # 64 row tiles of (128, 256)
MB = MROWS // P          # 8 m-blocks per core
QUAD = 2048              # psum quad width (4 banks)
NQ = R // QUAD           # 4 quads per block row
CH = 8                   # preprocessing chunks (8 row-tiles = 1024 rows each)
TPG = NT_ROW // CH
KC = D // P              # 2 contraction chunks of 128
BIG = 30000.0            # diag mask subtrahend (cos <= 1)

_CACHE = {}


def _host_constants():
    ident = np.eye(P, dtype=np.float32)
    bigI = (np.eye(P) * BIG).astype(np.float32)
    return {"ident_f": ident, "bigI": bigI}


def _build_nc():
    from contextlib import ExitStack

    import concourse.bass as bass
    import concourse.mybir as mybir
    import concourse.tile as tile
    from concourse import bacc

    f32 = mybir.dt.float32
    bf16 = mybir.dt.bfloat16
    AF = mybir.ActivationFunctionType
    X = mybir.AxisListType.X

    nc = bacc.Bacc(
        "TRN2",
        target_bir_lowering=False,
        debug=False,
        enable_asserts=False,
        num_devices=NCORES,
    )

    z_dram = nc.dram_tensor("z", [R, D], f32, kind="ExternalInput")
    ident_dram = nc.dram_tensor("ident_f", [P, P], f32, kind="ExternalInput")
    bigI_dram = nc.dram_tensor("bigI", [P, P], f32, kind="ExternalInput")
    rowmax_dram = nc.dram_tensor("row_max", [P, MB], f32, kind="ExternalOutput")
    pos_dram = nc.dram_tensor("pos", [P, MB], f32, kind="ExternalOutput")

    with tile.TileContext(nc) as tc, ExitStack() as ctx:
        singles = ctx.enter_context(tc.tile_pool(name="singles", bufs=1))
        big = ctx.enter_context(tc.tile_pool(name="big", bufs=1))
        zf_pool = ctx.enter_context(tc.tile_pool(name="zf_pool", bufs=3))
        sq_pool = ctx.enter_context(tc.tile_pool(name="sq_pool", bufs=2))
        cand_pool = ctx.enter_context(tc.tile_pool(name="cand_pool", bufs=2))
        fold_pool = ctx.enter_context(tc.tile_pool(name="fold_pool", bufs=2))
        scr_pool = ctx.enter_context(tc.tile_pool(name="scr_pool", bufs=2))
        dram = ctx.enter_context(
            tc.tile_pool(name="dram", bufs=1, space=bass.MemorySpace.DRAM)
        )
        psum = ctx.enter_context(
            tc.tile_pool(name="psum", bufs=2, space=bass.MemorySpace.PSUM)
        )

        # --- constants (loaded from host) ---
        ident_f = singles.tile([P, P], f32)
        nc.sync.dma_start(out=ident_f, in_=ident_dram.ap())
        bigI = singles.tile([P, P], f32)
        nc.sync.dma_start(out=bigI, in_=bigI_dram.ap())

        # --- persistent buffers ---
        zb = big.tile([P, NT_ROW, D], bf16)     # row-major normalized bf16
        zT0 = big.tile([P, R], bf16)            # [d 0:128, row]
        zT1 = big.tile([P, R], bf16)            # [d 128:256, row]
        zT = [zT0, zT1]
        n2 = singles.tile([P, NT_ROW], f32)
        nrm = singles.tile([P, NT_ROW], f32)
        inv = singles.tile([P, NT_ROW], f32)
        rowmax_sb = singles.tile([P, MB], f32)
        pos_sb = singles.tile([P, MB], f32)
        znb_d = dram.tile([R, D], bf16)         # DRAM scratch for transpose

        z_src = z_dram.ap().rearrange("(t p) d -> p t d", p=P)

        # --- preprocessing, per chunk of 1024 rows ---
        for g in range(CH):
            gs = slice(g * TPG, (g + 1) * TPG)
            zf = zf_pool.tile([P, TPG, D], f32, name="zf")
            nc.gpsimd.dma_start(out=zf, in_=z_src[:, gs, :])
            sq = sq_pool.tile([P, TPG, D], f32, name="sq")
            nc.scalar.activation(out=sq, in_=zf, func=AF.Square)
            nc.vector.reduce_sum(out=n2[:, gs], in_=sq, axis=X)
            nc.scalar.activation(out=nrm[:, gs], in_=n2[:, gs], func=AF.Sqrt)
            nc.vector.reciprocal(out=inv[:, gs], in_=nrm[:, gs])
            for j in range(TPG):
                t = g * TPG + j
                # fused scale + downcast (DVE tensor_scalar, f32 2x mode)
                nc.vector.tensor_scalar_mul(
                    zb[:, t, :], zf[:, j, :], inv[:, t : t + 1]
                )
            nc.sync.dma_start(
                out=znb_d[g * MROWS : (g + 1) * MROWS, :].rearrange(
                    "(j p) d -> p j d", p=P
                ),
                in_=zb[:, gs, :],
            )
            # xbar transpose loads: [1024, 128] DRAM -> [128, 1024] SBUF
            for c in range(KC):
                nc.sync.dma_start(
                    out=zT[c][:, g * MROWS : (g + 1) * MROWS],
                    in_=znb_d[g * MROWS : (g + 1) * MROWS, c * P : (c + 1) * P],
                    transpose=True,
                )

        # --- main: per 128-row block, 4 psum quads over all 8192 columns ---
        prev = None  # deferred fold state: (cand, block_index)
        for b in range(MB):
            o = b * P
            cand = cand_pool.tile([P, R], bf16, name="cand")
            for q in range(NQ):
                pp = psum.tile([P, QUAD], f32, name="pp")
                for c in range(KC):
                    for u in range(QUAD // 512):
                        col = q * QUAD + u * 512
                        nc.tensor.matmul(
                            pp[:, u * 512 : (u + 1) * 512],
                            zT[c][:, o : o + P],
                            zT[c][:, col : col + 512],
                            start=(c == 0),
                            stop=(c == KC - 1),
                        )
                if q == 0:
                    # mask self-similarity (diag block at columns o..o+128)
                    nc.vector.tensor_sub(
                        pp[:, o : o + P], pp[:, o : o + P], bigI
                    )
                if q == 2:
                    # positives: diag of the block at columns 4096+o
                    scr = scr_pool.tile([P, P], f32, name="scr")
                    nc.vector.tensor_mul(scr, pp[:, o : o + P], ident_f)
                    nc.vector.reduce_sum(out=pos_sb[:, b : b + 1], in_=scr, axis=X)
                # evacuate quad to bf16 candidates (ACT)
                nc.scalar.copy(out=cand[:, q * QUAD : (q + 1) * QUAD], in_=pp[:])
            # deferred deep fold of the PREVIOUS block (so the DVE never gates
            # this block's psum ops)
            if prev is not None:
                _fold(nc, tc, fold_pool, prev[0], rowmax_sb, prev[1], bf16, X)
            prev = (cand, b)
        _fold(nc, tc, fold_pool, prev[0], rowmax_sb, prev[1], bf16, X)

        nc.sync.dma_start(out=rowmax_dram.ap(), in_=rowmax_sb[:])
        nc.sync.dma_start(out=pos_dram.ap(), in_=pos_sb[:])

    nc.compile()
    return nc


def _fold(nc, tc, fold_pool, cand, rowmax_sb, b, bf16, X):
    P_ = P
    w = fold_pool.tile([P_, R // 2], bf16, name="w")
    nc.vector.tensor_max(w[:], cand[:, : R // 2], cand[:, R // 2 :])
    nc.vector.tensor_max(w[:, :2048], w[:, :2048], w[:, 2048:4096])
    nc.vector.tensor_max(w[:, :1024], w[:, :1024], w[:, 1024:2048])
    nc.vector.tensor_max(w[:, :512], w[:, :512], w[:, 512:1024])
    nc.vector.reduce_max(out=rowmax_sb[:, b : b + 1], in_=w[:, :512], axis=X)


def _get_nc():
    if "nc" not in _CACHE:
        _CACHE["nc"] = _build_nc()
    return _CACHE["nc"]


def _finish(rowmax_all: np.ndarray, pos_all: np.ndarray) -> np.ndarray:
    negmax = rowmax_all.astype(np.float64) / TEMPERATURE
    pos = pos_all.astype(np.float64) / TEMPERATURE
    m = pos.max()
    lse = np.log(np.exp(pos - m).sum()) + m
    return np.array(negmax.mean() - lse, dtype=np.float32)


def kernel(z_i: np.ndarray, z_j: np.ndarray, _collect=None, _run_kwargs=None) -> np.ndarray:
    from concourse.bass_utils import run_bass_kernel_spmd

    z_full = np.concatenate(
        [np.asarray(z_i, np.float32), np.asarray(z_j, np.float32)], axis=0
    )
    consts = _host_constants()
    in_maps = [
        {"z": np.ascontiguousarray(np.roll(z_full, -k * MROWS, axis=0)), **consts}
        for k in range(NCORES)
    ]
    nc = _get_nc()
    res = run_bass_kernel_spmd(
        nc, in_maps, core_ids=list(range(NCORES)), **(_run_kwargs or {})
    )
    if _collect is not None:
        _collect.append(res)
    rowmax_all = np.concatenate(
        [r["row_max"].T.reshape(-1) for r in res.results]
    )  # (8192,) in original row order
    pos_all = np.concatenate([r["pos"].T.reshape(-1) for r in res.results])
    return _finish(rowmax_all, pos_all)
